# revision 4
# baseline (speedup 1.0000x reference)
"""TRN2 Bass/Tile kernel for nn_AttentionTemporalEncoder (B=32, H=1024, T=512, 16 heads).

Strategy: data-parallel over batch - 4 batches per NeuronCore on 8 cores,
weights replicated to every core.  Per batch, entirely on-chip:

  Key compaction: the key-padding mask zeroes ~half the keys exactly
  (masked scores -> exp(-1e4) = 0), so the host gathers only the unmasked
  key columns of x per batch (TK_b ~ 256 of 512) and pads to the global
  max TKP; K-proj, V-proj, scores and attnV then run over ceil(TKP/128)
  key chunks instead of 4 - PE matmul time on TRN2 is (free-dim columns)
  x cycles_per_row, independent of partial partition dims.  Pad columns
  are zero and carry a -1e4 exp-bias (per-batch data, keeping the program
  SPMD-identical across cores).

  Qt = (Wq @ X + bq)/8 hidden-major [H, T] over all 512 queries; Q is
  stored as two zero-padded variants (even heads in partition rows 0-63,
  odd heads in 64-127) so every scores matmul contracts over the full
  K=128 rows.  Kt = Wk @ Xk + bk is [H, TKP].
  V = Xk.T @ Wv.T, key-major, with a ones-column per head so the attention
  matmul also emits the softmax denominators for free.
  Scores are computed transposed [TKP, Tq] in KC chunks; the pad-key bias
  (-1e4) is folded into the Exp activation (exp -> exact 0); no
  max-subtraction needed (|scores| <~ 3).
  Oraw_h = (V_h.T @ exp(scores^T)) * (1/denominators)  (fast DVE reciprocal,
  gpsimd partition-broadcast).
  y^T = Wo @ Oraw emitted transposed so the final max over T is a free-dim
  reduce_max.  Host folds 1/sqrt(64) into Wq/bq and adds Wo@bv + bo after
  the gather (max over T commutes with per-channel constants).

Scheduling: the PE instruction stream is emission order, so the emitter is
a fine-grained scheduler.  Each scores matmul (whose Exp consumer on the
scalar engine is ~3x slower than the matmul itself) is woven 1:1 with an
attnV matmul of the previous head-pair plus one 8-matmul "filler" unit
(Q/K/V projection chunk of an upcoming batch, or an output-projection
chunk of the previous batch) drained FIFO from a queue, so the PE never
waits on the exp chain or the normalize chain.  Weights are repacked
host-side into 128x128 (oc, ic) blocks so the first projection matmuls
depend only on a 256 KB DMA, not the full 2 MB weight tile.  Compute dtype
fp16 (fp32 PSUM accumulation).
"""

import os
import sys
from collections import deque

import numpy as np

for _p in ("/opt/trn_rl_repo", "/opt/pypackages"):
    if os.path.isdir(_p) and _p not in sys.path:
        sys.path.append(_p)

import concourse.tile as tile
from concourse import bacc, mybir

F32 = mybir.dt.float32
DT = mybir.dt.float16
NP_DT = np.float16

B, H, T = 32, 1024, 512
NH, HD = 16, 64
N_CORES = 8
NB = B // N_CORES  # batches per core
OC = H // 128      # hidden chunks
TC = T // 128      # token chunks (queries)
NP2 = NH // 2      # head pairs


def _build_nc(tkp):
    dt = DT
    nb = NB
    kc_n = (tkp + 127) // 128          # key chunks
    mks = [min(128, tkp - 128 * kc) for kc in range(kc_n)]  # rows per chunk
    nc = bacc.Bacc("TRN2", target_bir_lowering=False, debug=False, num_devices=N_CORES)

    x_d = nc.dram_tensor("x", [nb, OC, 128, T], dt, kind="ExternalInput")
    xk_d = nc.dram_tensor("xk", [nb, OC, 128, tkp], dt, kind="ExternalInput")
    # weights in block layout: [oc, p, ic, c] so one DMA delivers the 8
    # 128x128 (ic) blocks that produce output chunk oc
    wq_d = nc.dram_tensor("wq", [OC, 128, OC, 128], dt, kind="ExternalInput")
    wk_d = nc.dram_tensor("wk", [OC, 128, OC, 128], dt, kind="ExternalInput")
    wv_d = nc.dram_tensor("wv", [2, 128, OC, 512], dt, kind="ExternalInput")
    wo_d = nc.dram_tensor("wo", [OC, 128, OC, 128], dt, kind="ExternalInput")
    bq_d = nc.dram_tensor("bq", [OC, 128], F32, kind="ExternalInput")
    bk_d = nc.dram_tensor("bk", [OC, 128], F32, kind="ExternalInput")
    mb_d = nc.dram_tensor("maskbias", [nb, kc_n, 128], F32, kind="ExternalInput")
    y_d = nc.dram_tensor("y", [nb, OC, 128], F32, kind="ExternalOutput")

    from contextlib import ExitStack

    with tile.TileContext(nc) as tc, ExitStack() as ctx:
        consts = ctx.enter_context(tc.tile_pool(name="consts", bufs=1))
        xpool = ctx.enter_context(tc.tile_pool(name="xpool", bufs=4))
        qkv = ctx.enter_context(tc.tile_pool(name="qkv", bufs=2))
        attnp = ctx.enter_context(tc.tile_pool(name="attnp", bufs=4))
        acts = ctx.enter_context(tc.tile_pool(name="acts", bufs=1))
        smallp = ctx.enter_context(tc.tile_pool(name="smallp", bufs=4))
        resp = ctx.enter_context(tc.tile_pool(name="resp", bufs=2))
        ps_proj = ctx.enter_context(tc.tile_pool(name="ps_proj", bufs=3, space="PSUM"))
        ps_sc = ctx.enter_context(tc.tile_pool(name="ps_sc", bufs=3, space="PSUM"))
        ps_ao = ctx.enter_context(tc.tile_pool(name="ps_ao", bufs=2, space="PSUM"))

        # ---- tiles ----
        bq_sb = consts.tile([128, OC], F32, tag="bq")
        bk_sb = consts.tile([128, OC], F32, tag="bk")
        mb_sb = consts.tile([128, nb * kc_n], F32, tag="mb")
        wq_sb = consts.tile([128, OC, OC, 128], dt, tag="wq")  # [p, oc, ic, c]
        wk_sb = consts.tile([128, OC, OC, 128], dt, tag="wk")
        wv_sb = consts.tile([128, 2, OC, 512], dt, tag="wv")   # [p, ch, ic, c]
        wo_sb = consts.tile([128, OC, OC, 128], dt, tag="wo")
        x_tiles = [xpool.tile([128, OC, T], dt, tag="x", name=f"x{b}") for b in range(nb)]
        xk_tiles = [xpool.tile([128, OC, tkp], dt, tag="xk", name=f"xk{b}") for b in range(nb)]

        # ---- DMA emission, ordered by first consumer ----
        for ic in range(OC):
            nc.sync.dma_start(out=x_tiles[0][:, ic, :], in_=x_d.ap()[0, ic])
        nc.scalar.dma_start(out=wq_sb[:, 0], in_=wq_d.ap()[0])
        nc.scalar.dma_start(out=wk_sb[:, 0], in_=wk_d.ap()[0])
        for ic in range(OC):
            nc.sync.dma_start(out=xk_tiles[0][:, ic, :], in_=xk_d.ap()[0, ic])
        nc.scalar.dma_start(out=wq_sb[:, 1], in_=wq_d.ap()[1])
        nc.scalar.dma_start(out=wk_sb[:, 1], in_=wk_d.ap()[1])
        nc.gpsimd.dma_start(out=bq_sb[:, :], in_=bq_d.ap().rearrange("c p -> p c"))
        nc.gpsimd.dma_start(out=bk_sb[:, :], in_=bk_d.ap().rearrange("c p -> p c"))
        nc.gpsimd.dma_start(out=mb_sb[:, :], in_=mb_d.ap().rearrange("b c p -> p (b c)"))
        nc.gpsimd.dma_start(out=wv_sb[:, 0], in_=wv_d.ap()[0])
        for oc in range(2, OC):
            nc.scalar.dma_start(out=wq_sb[:, oc], in_=wq_d.ap()[oc])
            nc.scalar.dma_start(out=wk_sb[:, oc], in_=wk_d.ap()[oc])
        nc.gpsimd.dma_start(out=wv_sb[:, 1], in_=wv_d.ap()[1])
        for oc in range(OC):
            nc.gpsimd.dma_start(out=wo_sb[:, oc], in_=wo_d.ap()[oc])
        for b in range(1, nb):
            for ic in range(OC):
                nc.sync.dma_start(out=x_tiles[b][:, ic, :], in_=x_d.ap()[b, ic])
            for ic in range(OC):
                nc.sync.dma_start(out=xk_tiles[b][:, ic, :], in_=xk_d.ap()[b, ic])

        # persistent activation tiles, reused in place across batches (WAR
        # deps order next-batch writes after this batch's reads)
        qte_sb = acts.tile([128, OC, T], dt, tag="qte")
        qto_sb = acts.tile([128, OC, T], dt, tag="qto")
        kt_sb = acts.tile([128, OC, tkp], dt, tag="kt")
        v_sb = acts.tile([128, kc_n, NH, HD + 1], dt, tag="v")
        nc.vector.memset(v_sb[:, :, :, HD:HD + 1], 1.0)
        nc.vector.memset(qte_sb[64:128, :, :], 0.0)
        nc.vector.memset(qto_sb[0:64, :, :], 0.0)

        # ---- schedulable units (each ~8 matmuls + epilogue on DVE) ----
        def u_q(oc, x_sb):
            def run():
                ps = ps_proj.tile([128, T], F32, tag="proj", name="ps_q")
                for ic in range(OC):
                    nc.tensor.matmul(
                        ps[:, :],
                        lhsT=wq_sb[:, oc, ic, :],
                        rhs=x_sb[:, ic, :],
                        start=(ic == 0),
                        stop=(ic == OC - 1),
                    )
                nc.vector.tensor_scalar_add(qte_sb[0:64, oc, :], ps[0:64, :], bq_sb[0:64, oc:oc + 1])
                nc.vector.tensor_scalar_add(qto_sb[64:128, oc, :], ps[64:128, :], bq_sb[64:128, oc:oc + 1])
            return run

        def u_k(oc, xk_sb):
            def run():
                ps = ps_proj.tile([128, T], F32, tag="proj", name="ps_k")
                for ic in range(OC):
                    nc.tensor.matmul(
                        ps[:, 0:tkp],
                        lhsT=wk_sb[:, oc, ic, :],
                        rhs=xk_sb[:, ic, :],
                        start=(ic == 0),
                        stop=(ic == OC - 1),
                    )
                nc.vector.tensor_scalar_add(kt_sb[:, oc, :], ps[:, 0:tkp], bk_sb[:, oc:oc + 1])
            return run

        def u_v(ch, kcc, xk_sb):
            mk = mks[kcc]

            def run():
                ps = ps_proj.tile([128, T], F32, tag="proj", name="ps_v")
                for ic in range(OC):
                    nc.tensor.matmul(
                        ps[0:mk, :],
                        lhsT=xk_sb[:, ic, 128 * kcc:128 * kcc + mk],
                        rhs=wv_sb[:, ch, ic, :],
                        start=(ic == 0),
                        stop=(ic == OC - 1),
                    )
                nc.vector.tensor_copy(
                    out=v_sb[0:mk, kcc, 8 * ch:8 * (ch + 1), 0:HD],
                    in_=ps[0:mk, :].rearrange("p (h d) -> p h d", h=8),
                )
            return run

        def u_yp(res_sb, outraw_sb, oc, dma_b=None):
            def run():
                ps = ps_proj.tile([128, T], F32, tag="proj", name="ps_y")
                for cc in range(OC):
                    nc.tensor.matmul(
                        ps[:, :],
                        lhsT=wo_sb[:, oc, cc, :],
                        rhs=outraw_sb[:, cc, :],
                        start=(cc == 0),
                        stop=(cc == OC - 1),
                    )
                nc.vector.reduce_max(res_sb[:, oc:oc + 1], ps[:, :], axis=mybir.AxisListType.X)
                if dma_b is not None:
                    nc.gpsimd.dma_start(
                        out=y_d.ap()[dma_b].rearrange("c p -> p c"), in_=res_sb[:, :]
                    )
            return run

        fillq = deque()

        def fill(n=1):
            for _ in range(n):
                if fillq:
                    fillq.popleft()()

        outraw_tiles = {}
        res_tiles = {}

        # scores matmul i (kc, head-of-pair) of pair hp, plus its Exp
        def emit_s(b, hp, i, attns):
            kcc, hi = divmod(i, 2)
            mk = mks[kcc]
            h = 2 * hp + hi
            q_sb = qte_sb if h % 2 == 0 else qto_sb
            ps_s = ps_sc.tile([128, T], F32, tag="sc")
            nc.tensor.matmul(
                ps_s[0:mk, :],
                lhsT=kt_sb[:, hp, 128 * kcc:128 * kcc + mk],
                rhs=q_sb[:, hp, :],
                start=True,
                stop=True,
            )
            nc.scalar.activation(
                attns[hi][0:mk, kcc, :],
                ps_s[0:mk, :],
                mybir.ActivationFunctionType.Exp,
                bias=mb_sb[0:mk, b * kc_n + kcc:b * kc_n + kcc + 1],
                scale=1.0,
            )

        # attnV matmul i of pair hp; on the last chunk of each head, the
        # normalize epilogue (denom reciprocal broadcast + scale)
        def emit_a(b, hp, i, attns, ao_state):
            hi, kcc = divmod(i, kc_n)
            mk = mks[kcc]
            h = 2 * hp + hi
            if kcc == 0:
                ao_state[hi] = ps_ao.tile([HD + 1, T], F32, tag="ao", name="ps_o")
            ps_o = ao_state[hi]
            nc.tensor.matmul(
                ps_o[:, :],
                lhsT=v_sb[0:mk, kcc, h, :],
                rhs=attns[hi][0:mk, kcc, :],
                start=(kcc == 0),
                stop=(kcc == kc_n - 1),
            )
            if kcc == kc_n - 1:
                hc, ho = h // 2, 64 * (h % 2)
                outraw_sb = outraw_tiles[b]
                sums1 = smallp.tile([1, T], F32, tag="sums1")
                nc.vector.tensor_copy(out=sums1[:, :], in_=ps_o[HD:HD + 1, :])
                recip1 = smallp.tile([1, T], F32, tag="recip1")
                nc.vector.reciprocal_approx_fast(recip1[:, :], sums1[:, :])
                recip64 = smallp.tile([64, T], F32, tag="recip64")
                nc.gpsimd.partition_broadcast(recip64[:, :], recip1[:, :])
                nc.vector.tensor_mul(outraw_sb[ho:ho + 64, hc, :], ps_o[0:HD, :], recip64[:, :])

        n_units = 2 * kc_n  # scores matmuls per pair == attnV matmuls per pair

        for b in range(nb):
            outraw_tiles[b] = qkv.tile([128, OC, T], dt, tag="outraw", name=f"outraw{b}")
            res_tiles[b] = resp.tile([128, OC], F32, tag="res", name=f"res{b}")

            if b == 0:
                # prologue: first two QK chunks + V half 0, emitted directly
                u_q(0, x_tiles[0])()
                u_k(0, xk_tiles[0])()
                u_q(1, x_tiles[0])()
                u_k(1, xk_tiles[0])()
                for kcc in range(kc_n):
                    u_v(0, kcc, xk_tiles[0])()

            prev_attns = None if b == 0 else batch_attns  # noqa: F821
            prev_ao = None if b == 0 else batch_ao  # noqa: F821

            for hp in range(NP2):
                # enqueue upcoming work at iteration start (FIFO)
                if hp + 2 < OC:
                    fillq.append(u_q(hp + 2, x_tiles[b]))
                    fillq.append(u_k(hp + 2, xk_tiles[b]))
                if hp == 3:
                    for kcc in range(kc_n):
                        fillq.append(u_v(1, kcc, xk_tiles[b]))
                if b + 1 < nb:
                    if hp == 4:
                        fillq.append(u_q(0, x_tiles[b + 1]))
                        fillq.append(u_k(0, xk_tiles[b + 1]))
                    if hp == 5:
                        fillq.append(u_q(1, x_tiles[b + 1]))
                        fillq.append(u_k(1, xk_tiles[b + 1]))
                    if hp == 6:
                        for kcc in range(kc_n):
                            fillq.append(u_v(0, kcc, xk_tiles[b + 1]))

                attns = (attnp.tile([128, kc_n, T], dt, tag="attn0", name="attn0"),
                         attnp.tile([128, kc_n, T], dt, tag="attn1", name="attn1"))
                ao_state = [None, None]
                # partner attnV: pair hp-1 of this batch, or pair 7 of b-1
                if hp > 0:
                    pa, pb, p_ao = prev_attns, b, prev_ao
                elif b > 0:
                    pa, pb, p_ao = prev_attns, b - 1, prev_ao
                else:
                    pa = None

                for i in range(n_units):
                    emit_s(b, hp, i, attns)
                    if pa is not None:
                        emit_a(pb, NP2 - 1 if hp == 0 else hp - 1, i, pa, p_ao)
                        if hp == 0 and i == n_units - 1:
                            # previous batch's outraw is complete: queue its
                            # output projection + result DMA
                            for oc in range(OC):
                                fillq.append(u_yp(
                                    res_tiles[b - 1], outraw_tiles[b - 1], oc,
                                    dma_b=b - 1 if oc == OC - 1 else None,
                                ))
                    fill(1)

                prev_attns, prev_ao = attns, ao_state

            batch_attns, batch_ao = prev_attns, prev_ao

        # epilogue: attnV of the last pair of the last batch, then its yproj
        bl = nb - 1
        for i in range(n_units):
            emit_a(bl, NP2 - 1, i, batch_attns, batch_ao)
            fill(1)
        while fillq:
            fill(1)
        for oc in range(OC):
            u_yp(res_tiles[bl], outraw_tiles[bl], oc,
                 dma_b=bl if oc == OC - 1 else None)()

    nc.compile()
    return nc


_NC_CACHE = {}


def _get_nc(tkp):
    if tkp not in _NC_CACHE:
        _NC_CACHE[tkp] = _build_nc(tkp)
    return _NC_CACHE[tkp]


def _prep(x, mask, Wq, bq, Wk, bk, Wv, bv, Wo, bo):
    """Host-side prep: fold scales, gather unmasked key columns, repack, shard."""
    x = np.asarray(x, dtype=np.float32)
    mask = np.asarray(mask)
    Wq, bq, Wk, bk, Wv, bv, Wo, bo = (
        np.asarray(a, dtype=np.float32) for a in (Wq, bq, Wk, bk, Wv, bv, Wo, bo)
    )
    scale = np.float32(1.0 / np.sqrt(np.float32(HD)))

    # block layouts [oc, p, ic, c]: W.T[ic*128+p, oc*128+c]
    def blocks(wt, csz):
        return np.ascontiguousarray(
            wt.reshape(OC, 128, H // csz, csz).transpose(2, 1, 0, 3).astype(NP_DT)
        )

    wq_b = blocks(Wq.T * scale, 128)
    wk_b = blocks(Wk.T, 128)
    wv_b = blocks(Wv.T, 512)
    wo_b = blocks(Wo.T, 128)
    bq_s = np.ascontiguousarray((bq * scale).reshape(OC, 128).astype(np.float32))
    bk_s = np.ascontiguousarray(bk.reshape(OC, 128).astype(np.float32))

    idx = [np.nonzero(mask[b] != 0)[0] for b in range(B)]
    tks = [len(i) for i in idx]
    tkp = max(tks)
    kc_n = (tkp + 127) // 128

    x16 = x.astype(NP_DT)
    xk = np.zeros((B, H, tkp), dtype=NP_DT)
    for b in range(B):
        xk[b, :, : tks[b]] = x16[b][:, idx[b]]
    maskbias = np.zeros((B, kc_n * 128), dtype=np.float32)
    for b in range(B):
        maskbias[b, tks[b]:] = np.float32(-10000.0)

    in_maps = []
    for c in range(N_CORES):
        sl = slice(c * NB, (c + 1) * NB)
        in_maps.append({
            "x": np.ascontiguousarray(x16[sl].reshape(NB, OC, 128, T)),
            "xk": np.ascontiguousarray(xk[sl].reshape(NB, OC, 128, tkp)),
            "wq": wq_b, "wk": wk_b, "wv": wv_b, "wo": wo_b,
            "bq": bq_s, "bk": bk_s,
            "maskbias": np.ascontiguousarray(maskbias[sl].reshape(NB, kc_n, 128)),
        })
    return in_maps, tkp


def kernel(x, mask, Wq, bq, Wk, bk, Wv, bv, Wo, bo):
    in_maps, tkp = _prep(x, mask, Wq, bq, Wk, bk, Wv, bv, Wo, bo)

    from concourse.bass_utils import run_bass_kernel_spmd

    nc = _get_nc(tkp)
    res = run_bass_kernel_spmd(nc, in_maps, core_ids=list(range(N_CORES)))
    y = np.concatenate(
        [res.results[i]["y"].reshape(NB, H) for i in range(N_CORES)], axis=0
    )
    # max over T commutes with the per-channel constant Wo @ bv + bo
    Wo = np.asarray(Wo, dtype=np.float32)
    bv = np.asarray(bv, dtype=np.float32)
    bo = np.asarray(bo, dtype=np.float32)
    bo2 = Wo @ bv + bo
    return (y + bo2[None, :]).astype(np.float32)
